# revision 10
# baseline (speedup 1.0000x reference)
"""Trainium2 Bass kernel for nn_DynamicsLookAheadModel.

LSTM warm-up over S=96 steps + 32-step look-ahead with output feedback,
data-parallel over the batch (2048) across 8 NeuronCores (256 per core).

Per-core layout (all fp32):
  - Everything "transposed": hidden units on partitions, batch on the free dim.
    H=256 tensors are folded into [128, 512] tiles:
      phys[p, j]       = logical[p,     j]   for j <  256   (h-dims 0..127)
      phys[p, 256 + j] = logical[128+p, j]                  (h-dims 128..255)
  - Gates g = W_ih@x + b + W_hh@h computed on the PE into PSUM, one bank per
    gate, as out = lhsT.T @ rhs with K-tiles {33 (x plus ones-row bias), 128,
    128 (folded h halves)}.
  - Sigmoid/Tanh on ScalarE straight from PSUM; cell update on VectorE.
  - STE binarization uses sign(c') (sigmoid(o) > 0 always, tanh sign-preserving),
    so bits = (c' > 0) via one tensor_scalar is_gt.
  - Outputs are stored per step as [6, 256] and assembled as [33, 6, 256] in
    DRAM; the host gather transposes to [256, 33, 6].
"""

import os

import numpy as np

import concourse.bass as bass
import concourse.bass_utils as bass_utils
import concourse.mybir as mybir
import concourse.tile as tile
from concourse.bass_utils import run_bass_kernel_spmd

# A/B toggle: let walrus optimize LDWEIGHTS scheduling (default off upstream).
if os.environ.get("LDW_OPT") == "1" and not getattr(bass_utils, "_ldw_patched", False):
    bass_utils._ldw_patched = True
    _orig_run_command = bass_utils.run_command

    def _run_command_ldw(argv, **kwargs):
        argv = [
            "--enable-ldw-opt=true" if a == "--enable-ldw-opt=false" else a
            for a in argv
        ]
        return _orig_run_command(argv, **kwargs)

    bass_utils.run_command = _run_command_ldw

B, S, F, H, O = 2048, 96, 32, 256, 6
LA = 32
NCORES = 8
BL = B // NCORES  # 256 per-core batch
FD = 2 * BL  # 512: folded free dim for H=256 tensors
KX = F + 1  # 33: x features + ones row (bias)
FP32 = mybir.dt.float32


# --- workaround: this walrus build allows only ONE sem wait per instruction ---
# Spill excess semaphore waits onto same-engine NOPs placed just before the
# instruction (engines execute their queue in order, so semantics match).
def _spill_excess_waits(nc, limit=1):
    cnt = 0
    for f in nc.m.functions:
        for bb in f.blocks:
            new_list = []
            for ins in bb.instructions:
                si = ins.sync_info
                if si and si.on_wait and len(si.on_wait) > limit:
                    waits = list(si.on_wait)
                    for w in waits[:-limit]:
                        n = mybir.InstNoOp(name=f"wspill_{cnt}", ins=[], outs=[])
                        cnt += 1
                        n.engine = ins.engine
                        n.sync_info = mybir.SyncInfo(on_wait=[w], on_update=[])
                        new_list.append(n)
                    ins.sync_info = mybir.SyncInfo(
                        on_wait=waits[-limit:], on_update=list(si.on_update)
                    )
                new_list.append(ins)
            bb.instructions[:] = new_list
    return cnt


def build_nc(n_warm=S, n_la=LA, spill=True):
    from contextlib import ExitStack

    nc = bass.Bass()
    AF = mybir.ActivationFunctionType
    ALU = mybir.AluOpType

    xaug_d = nc.dram_tensor("xaug", [n_warm, KX, BL], FP32, kind="ExternalInput")
    wiha_d = nc.dram_tensor("wiha", [KX, 4 * H], FP32, kind="ExternalInput")
    whh0_d = nc.dram_tensor("whh0", [128, 4 * H], FP32, kind="ExternalInput")
    whh1_d = nc.dram_tensor("whh1", [128, 4 * H], FP32, kind="ExternalInput")
    wfc_d = nc.dram_tensor("wfc", [128, 2 * O], FP32, kind="ExternalInput")
    bfc_d = nc.dram_tensor("bfc", [O, 1], FP32, kind="ExternalInput")
    out_d = nc.dram_tensor("out_t", [n_la + 1, O, BL], FP32, kind="ExternalOutput")

    with tile.TileContext(nc) as tc, ExitStack() as es:
        wp_ctx = es.enter_context(tc.tile_pool(name="weights", bufs=1))
        xp_ctx = es.enter_context(tc.tile_pool(name="xtiles", bufs=1))
        sp_ctx = es.enter_context(tc.tile_pool(name="state", bufs=2))
        gp_ctx = es.enter_context(tc.tile_pool(name="gates", bufs=1, space="PSUM"))
        op_ctx = es.enter_context(tc.tile_pool(name="outp", bufs=2, space="PSUM"))

        # weights
        wiha = wp_ctx.tile([KX, 4 * H], FP32, tag="wiha")
        nc.sync.dma_start(out=wiha, in_=wiha_d[:, :])
        whh0 = wp_ctx.tile([128, 4 * H], FP32, tag="whh0")
        nc.sync.dma_start(out=whh0, in_=whh0_d[:, :])
        whh1 = wp_ctx.tile([128, 4 * H], FP32, tag="whh1")
        nc.sync.dma_start(out=whh1, in_=whh1_d[:, :])
        wfc = wp_ctx.tile([128, 2 * O], FP32, tag="wfc")
        nc.sync.dma_start(out=wfc, in_=wfc_d[:, :])
        bfc = wp_ctx.tile([O, 1], FP32, tag="bfc")
        nc.sync.dma_start(out=bfc, in_=bfc_d[:, :])

        # x tiles (one per step; LA reuses tiles 0..31 with rows 0:6 replaced)
        xt = []
        for t in range(n_warm):
            xtile = xp_ctx.tile([KX, BL], FP32, tag=f"x{t}")
            nc.sync.dma_start(out=xtile, in_=xaug_d[t, :, :])
            xt.append(xtile)

        # initial state
        c_prev = sp_ctx.tile([128, FD], FP32, tag="c")
        h_prev = None  # step 0 skips the W_hh matmuls entirely

        # gate order: g first (tanh feeds t2), then f, i, o
        GATES = [("g", 2), ("f", 1), ("i", 0), ("o", 3)]

        def lstm_step(xtile, h_prev, c_prev, first=False):
            ps = {}
            for name, gi in GATES:
                if first and name == "f":
                    continue  # sigmoid(f)*c is 0 at step 0
                p = gp_ctx.tile([128, FD], FP32, tag="p" + name)
                ps[name] = p
                for m in (0, 1):
                    col = gi * H + m * 128
                    osl = p[:, m * BL : (m + 1) * BL]
                    nc.tensor.matmul(
                        osl,
                        wiha[:, col : col + 128],
                        xtile[:, :],
                        start=True,
                        stop=first,
                    )
                    if not first:
                        nc.tensor.matmul(
                            osl,
                            whh0[:, col : col + 128],
                            h_prev[:, 0:BL],
                            start=False,
                            stop=False,
                        )
                        nc.tensor.matmul(
                            osl,
                            whh1[:, col : col + 128],
                            h_prev[:, BL:FD],
                            start=False,
                            stop=True,
                        )

            tg = sp_ctx.tile([128, FD], FP32, tag="tg")
            nc.scalar.activation(out=tg, in_=ps["g"][:, :], func=AF.Tanh)
            if not first:
                sf = sp_ctx.tile([128, FD], FP32, tag="sf")
                nc.scalar.activation(out=sf, in_=ps["f"][:, :], func=AF.Sigmoid)
            si = sp_ctx.tile([128, FD], FP32, tag="si")
            nc.scalar.activation(out=si, in_=ps["i"][:, :], func=AF.Sigmoid)
            so = sp_ctx.tile([128, FD], FP32, tag="so")
            nc.scalar.activation(out=so, in_=ps["o"][:, :], func=AF.Sigmoid)

            c_new = sp_ctx.tile([128, FD], FP32, tag="c")
            if first:
                nc.vector.tensor_tensor(out=c_new, in0=si, in1=tg, op=ALU.mult)
            else:
                t1 = sp_ctx.tile([128, FD], FP32, tag="t1")
                nc.vector.tensor_tensor(out=t1, in0=sf, in1=c_prev, op=ALU.mult)
                t2 = sp_ctx.tile([128, FD], FP32, tag="t2")
                nc.vector.tensor_tensor(out=t2, in0=si, in1=tg, op=ALU.mult)
                nc.vector.tensor_tensor(out=c_new, in0=t1, in1=t2, op=ALU.add)
            tc_t = sp_ctx.tile([128, FD], FP32, tag="tc")
            nc.scalar.activation(out=tc_t, in_=c_new, func=AF.Tanh)
            h_new = sp_ctx.tile([128, FD], FP32, tag="h")
            nc.vector.tensor_tensor(out=h_new, in0=so, in1=tc_t, op=ALU.mult)
            return h_new, c_new

        def emit_output(k, c_cur):
            # bits = (c' > 0); equals STE(h) since sigmoid(o)>0, tanh sign-pres.
            bits = sp_ctx.tile([128, FD], FP32, tag="bits")
            nc.vector.tensor_scalar(
                out=bits, in0=c_cur, scalar1=0.0, scalar2=None, op0=ALU.is_gt
            )
            po = op_ctx.tile([O, BL], FP32, tag="po")
            nc.tensor.matmul(
                po, wfc[:, 0:O], bits[:, 0:BL], start=True, stop=False
            )
            nc.tensor.matmul(
                po,
                wfc[:, O : 2 * O],
                bits[:, BL:FD],
                start=False,
                stop=True,
            )
            osb = sp_ctx.tile([O, BL], FP32, tag="osb")
            nc.scalar.activation(out=osb, in_=po, func=AF.Identity, bias=bfc)
            nc.sync.dma_start(out=out_d[k, :, :], in_=osb)
            return osb

        # warm-up
        for t in range(n_warm):
            h_prev, c_prev = lstm_step(xt[t], h_prev, c_prev, first=(t == 0))

        # look-ahead: output k uses c' of the step just computed; feed into
        # x tile k (rows 0:6) consumed by LA step k.
        for k in range(n_la + 1):
            osb = emit_output(k, c_prev)
            if k < n_la:
                nc.vector.tensor_copy(out=xt[k][0:O, :], in_=osb)
                h_prev, c_prev = lstm_step(xt[k], h_prev, c_prev)

    if spill:
        _spill_excess_waits(nc)
    return nc


def _host_prep(x, W_ih, W_hh, b_ih, b_hh, W_fc, b_fc):
    """Build the 8 per-core input maps."""
    bias = (b_ih + b_hh).astype(np.float32)
    wiha = np.concatenate([W_ih, bias[:, None]], axis=1).T  # [33, 1024]
    whh_t = np.ascontiguousarray(W_hh.T)  # [256, 1024]
    wfc_fold = np.concatenate([W_fc.T[:128], W_fc.T[128:]], axis=1)  # [128, 12]
    shared = {
        "wiha": np.ascontiguousarray(wiha).astype(np.float32),
        "whh0": np.ascontiguousarray(whh_t[:128]).astype(np.float32),
        "whh1": np.ascontiguousarray(whh_t[128:]).astype(np.float32),
        "wfc": np.ascontiguousarray(wfc_fold).astype(np.float32),
        "bfc": np.ascontiguousarray(b_fc.reshape(O, 1)).astype(np.float32),
    }
    ones = np.ones((S, 1, BL), dtype=np.float32)
    in_maps = []
    for c in range(NCORES):
        xc = x[c * BL : (c + 1) * BL]  # [BL, S, F]
        xT = np.ascontiguousarray(xc.transpose(1, 2, 0)).astype(np.float32)
        xaug = np.concatenate([xT, ones], axis=1)  # [S, 33, BL]
        in_maps.append({"xaug": np.ascontiguousarray(xaug), **shared})
    return in_maps


_NC_CACHE = {}


def _get_nc():
    if "nc" not in _NC_CACHE:
        _NC_CACHE["nc"] = build_nc()
    return _NC_CACHE["nc"]


def run(inputs, trace=False):
    in_maps = _host_prep(**inputs)
    nc = _get_nc()
    res = run_bass_kernel_spmd(nc, in_maps, core_ids=list(range(NCORES)), trace=trace)
    outs = []
    for c in range(NCORES):
        o = res.results[c]["out_t"]  # [33, 6, BL]
        outs.append(np.ascontiguousarray(o.transpose(2, 0, 1)))  # [BL, 33, 6]
    full = np.concatenate(outs, axis=0).astype(np.float32)  # [B, 33, 6]
    return full, res


def kernel(**inputs):
    full, _ = run(inputs, trace=False)
    return full


if __name__ == "__main__":
    t = build_nc()
    print("built ok")


# revision 14
# speedup vs baseline: 1.5316x; 1.5316x over previous
"""Trainium2 Bass kernel for nn_DynamicsLookAheadModel.

LSTM warm-up over S=96 steps + 32-step look-ahead with output feedback,
data-parallel over the batch (2048) across 8 NeuronCores (256 per core).

Per-core layout (all fp32):
  - Everything "transposed": hidden units on partitions, batch on the free dim.
    H=256 tensors are folded into [128, 512] tiles:
      phys[p, j]       = logical[p,     j]   for j <  256   (h-dims 0..127)
      phys[p, 256 + j] = logical[128+p, j]                  (h-dims 128..255)
  - Gates g = W_ih@x + W_hh@h computed on the PE into PSUM; the K=32
    x-projection runs as 4 concurrent quadrant matmuls (tile_position row
    packing, x and W_ih replicated across the 4 row bands), the K=256 h part
    as 2 full K=128 accumulation matmuls per M-tile.
  - PSUM: gate M-tile m (of 8) lives in bank m%4, columns 256*(m//4).
  - Bias (b_ih+b_hh) applied via the ScalarE activation bias port (per
    partition), one Sigmoid/Tanh instruction per M-tile from PSUM.
  - Cell update on VectorE in fp32.
  - STE binarization uses sign(c') (sigmoid(o) > 0, tanh sign-preserving):
    bits = (c' > 0) via one tensor_scalar is_gt.
  - Outputs are stored per step as [6, 256], assembled as [33, 6, 256] in
    DRAM; the host gather transposes to [256, 33, 6].
"""

import os

import numpy as np

import concourse.bass as bass
import concourse.mybir as mybir
import concourse.tile as tile
from concourse.bass_utils import run_bass_kernel_spmd

B, S, F, H, O = 2048, 96, 32, 256, 6
LA = 32
NCORES = 8
BL = B // NCORES  # 256 per-core batch
FD = 2 * BL  # 512: folded free dim for H=256 tensors
FP32 = mybir.dt.float32
# Matmul operand dtype: fp32 required — 16-bit quantization noise gets
# amplified by the STE binarization to several percent output error (measured).
MM16 = os.environ.get("MM16", "0") == "1"
MMDT = mybir.dt.float16 if MM16 else FP32
MMNP = np.float16 if MM16 else np.float32


# --- workaround: this walrus build allows only ONE sem wait per instruction ---
# Spill excess semaphore waits onto same-engine NOPs placed just before the
# instruction (engines execute their queue in order, so semantics match).
def _spill_excess_waits(nc, limit=1):
    cnt = 0
    for f in nc.m.functions:
        for bb in f.blocks:
            new_list = []
            for ins in bb.instructions:
                si = ins.sync_info
                if si and si.on_wait and len(si.on_wait) > limit:
                    waits = list(si.on_wait)
                    for w in waits[:-limit]:
                        n = mybir.InstNoOp(name=f"wspill_{cnt}", ins=[], outs=[])
                        cnt += 1
                        n.engine = ins.engine
                        n.sync_info = mybir.SyncInfo(on_wait=[w], on_update=[])
                        new_list.append(n)
                    ins.sync_info = mybir.SyncInfo(
                        on_wait=waits[-limit:], on_update=list(si.on_update)
                    )
                new_list.append(ins)
            bb.instructions[:] = new_list
    return cnt


def build_nc(n_warm=S, n_la=LA, spill=True):
    from contextlib import ExitStack

    nc = bass.Bass()
    AF = mybir.ActivationFunctionType
    ALU = mybir.AluOpType

    # xrep: x transposed per step and replicated over the 4 PE row bands.
    xrep_d = nc.dram_tensor("xrep", [n_warm, 128, BL], MMDT, kind="ExternalInput")
    # wrep: W_ih.T [32, 1024] replicated over the 4 row bands -> [128, 1024]
    wrep_d = nc.dram_tensor("wrep", [128, 4 * H], MMDT, kind="ExternalInput")
    whh0_d = nc.dram_tensor("whh0", [128, 4 * H], MMDT, kind="ExternalInput")
    whh1_d = nc.dram_tensor("whh1", [128, 4 * H], MMDT, kind="ExternalInput")
    wfc_d = nc.dram_tensor("wfc", [128, 2 * O], MMDT, kind="ExternalInput")
    bias8_d = nc.dram_tensor("bias8", [128, 8], FP32, kind="ExternalInput")
    bfc_d = nc.dram_tensor("bfc", [O, 1], FP32, kind="ExternalInput")
    out_d = nc.dram_tensor("out_t", [n_la + 1, O, BL], FP32, kind="ExternalOutput")

    with tile.TileContext(nc) as tc, ExitStack() as es:
        wp_ctx = es.enter_context(tc.tile_pool(name="weights", bufs=1))
        xp_ctx = es.enter_context(tc.tile_pool(name="xtiles", bufs=1))
        sp_ctx = es.enter_context(tc.tile_pool(name="state", bufs=2))
        gp_ctx = es.enter_context(tc.tile_pool(name="gates", bufs=1, space="PSUM"))

        wrep = wp_ctx.tile([128, 4 * H], MMDT, tag="wrep")
        nc.sync.dma_start(out=wrep, in_=wrep_d[:, :])
        whh0 = wp_ctx.tile([128, 4 * H], MMDT, tag="whh0")
        nc.sync.dma_start(out=whh0, in_=whh0_d[:, :])
        whh1 = wp_ctx.tile([128, 4 * H], MMDT, tag="whh1")
        nc.sync.dma_start(out=whh1, in_=whh1_d[:, :])
        wfc = wp_ctx.tile([128, 2 * O], MMDT, tag="wfc")
        nc.sync.dma_start(out=wfc, in_=wfc_d[:, :])
        bias8 = wp_ctx.tile([128, 8], FP32, tag="bias8")
        nc.sync.dma_start(out=bias8, in_=bias8_d[:, :])
        bfc = wp_ctx.tile([O, 1], FP32, tag="bfc")
        nc.sync.dma_start(out=bfc, in_=bfc_d[:, :])

        # x tiles (one per step; LA reuses tiles 0..31 with band rows 0:6
        # replaced by the fed-back output)
        xt = []
        for t in range(n_warm):
            xtile = xp_ctx.tile([128, BL], MMDT, tag=f"x{t}")
            nc.sync.dma_start(out=xtile, in_=xrep_d[t, :, :])
            xt.append(xtile)

        c_prev = sp_ctx.tile([128, FD], FP32, tag="c")
        h_prev = None  # step 0 skips the W_hh matmuls entirely

        # gate order i,f,g,o over M-tiles m=0..7 (gate X -> tiles 2X, 2X+1)
        # PSUM: one bank per M-tile (separate accumulation state per tile)
        GATE_FUNC = [AF.Sigmoid, AF.Sigmoid, AF.Tanh, AF.Sigmoid]  # i, f, g, o
        EMIT_ORDER = [2, 1, 0, 3]  # g, f, i, o

        def lstm_step(xtile, h_prev, c_prev, first=False):
            banks = []
            for b in range(8):
                pbank = gp_ctx.tile([128, BL], FP32, tag=f"pb{b}")
                banks.append(pbank)

            def psl(m):
                return banks[m][:, :]

            # x-projection: two quadrant-packed groups of 4 K=32 matmuls
            for grp in (0, 1):
                for band in range(4):
                    m = 4 * grp + band
                    if first and m in (2, 3):
                        continue  # f gate unused at step 0
                    nc.tensor.matmul(
                        psl(m),
                        wrep[32 * band : 32 * band + 32, 128 * m : 128 * m + 128],
                        xtile[32 * band : 32 * band + 32, :],
                        start=True,
                        stop=first,
                        tile_position=(32 * band, 0),
                        skip_group_check=True,
                    )
            # h part: per emission-ordered gate, 2 M-tiles x 2 K-tiles
            if not first:
                for g in EMIT_ORDER:
                    for m in (2 * g, 2 * g + 1):
                        col = 128 * m
                        nc.tensor.matmul(
                            psl(m),
                            whh0[:, col : col + 128],
                            h_prev[:, 0:BL],
                            start=False,
                            stop=False,
                            skip_group_check=True,
                        )
                        nc.tensor.matmul(
                            psl(m),
                            whh1[:, col : col + 128],
                            h_prev[:, BL:FD],
                            start=False,
                            stop=True,
                            skip_group_check=True,
                        )

            # activations: one instr per M-tile, bias via the ACT bias port
            act = {}
            for g in EMIT_ORDER:
                if first and g == 1:
                    continue
                a = sp_ctx.tile([128, FD], FP32, tag=f"a{g}")
                act[g] = a
                for half in (0, 1):
                    m = 2 * g + half
                    nc.scalar.activation(
                        out=a[:, 256 * half : 256 * half + 256],
                        in_=psl(m),
                        func=GATE_FUNC[g],
                        bias=bias8[:, m : m + 1],
                    )

            c_new = sp_ctx.tile([128, FD], FP32, tag="c")
            if first:
                nc.vector.tensor_tensor(out=c_new, in0=act[0], in1=act[2], op=ALU.mult)
            else:
                t1 = sp_ctx.tile([128, FD], FP32, tag="t1")
                nc.vector.tensor_tensor(out=t1, in0=act[1], in1=c_prev, op=ALU.mult)
                t2 = sp_ctx.tile([128, FD], FP32, tag="t2")
                nc.vector.tensor_tensor(out=t2, in0=act[0], in1=act[2], op=ALU.mult)
                nc.vector.tensor_tensor(out=c_new, in0=t1, in1=t2, op=ALU.add)
            tc_t = sp_ctx.tile([128, FD], FP32, tag="tc")
            nc.scalar.activation(out=tc_t, in_=c_new, func=AF.Tanh)
            h_new = sp_ctx.tile([128, FD], MMDT, tag="h")
            nc.vector.tensor_tensor(out=h_new, in0=act[3], in1=tc_t, op=ALU.mult)
            return h_new, c_new

        def emit_output(k, c_cur):
            # bits = (c' > 0); equals STE(h) since sigmoid(o)>0, tanh sign-pres.
            bits = sp_ctx.tile([128, FD], MMDT, tag="bits")
            nc.vector.tensor_scalar(
                out=bits, in0=c_cur, scalar1=0.0, scalar2=None, op0=ALU.is_gt
            )
            # reuse a gate bank slot: all gate reads of this step are done
            # before bits is ready, so the WAR dep is already satisfied
            po = gp_ctx.tile([O, BL], FP32, tag="pb0")
            nc.tensor.matmul(po, wfc[:, 0:O], bits[:, 0:BL], start=True, stop=False)
            nc.tensor.matmul(
                po, wfc[:, O : 2 * O], bits[:, BL:FD], start=False, stop=True
            )
            osb = sp_ctx.tile([O, BL], FP32, tag="osb")
            nc.scalar.activation(out=osb, in_=po, func=AF.Identity, bias=bfc)
            nc.sync.dma_start(out=out_d[k, :, :], in_=osb)
            return osb

        for t in range(n_warm):
            h_prev, c_prev = lstm_step(xt[t], h_prev, c_prev, first=(t == 0))

        for k in range(n_la + 1):
            osb = emit_output(k, c_prev)
            if k < n_la:
                for band in range(4):
                    nc.vector.tensor_copy(
                        out=xt[k][32 * band : 32 * band + O, :], in_=osb
                    )
                h_prev, c_prev = lstm_step(xt[k], h_prev, c_prev)

    if spill:
        _spill_excess_waits(nc)
    return nc


def _host_prep(x, W_ih, W_hh, b_ih, b_hh, W_fc, b_fc):
    """Build the 8 per-core input maps."""
    bias = (b_ih + b_hh).astype(np.float32)
    w32t = np.ascontiguousarray(W_ih.T).astype(MMNP)  # [32, 1024]
    wrep = np.ascontiguousarray(np.tile(w32t, (4, 1)))  # [128, 1024]
    whh_t = np.ascontiguousarray(W_hh.T)  # [256, 1024]
    wfc_fold = np.concatenate([W_fc.T[:128], W_fc.T[128:]], axis=1)  # [128, 12]
    shared = {
        "wrep": wrep,
        "whh0": np.ascontiguousarray(whh_t[:128]).astype(MMNP),
        "whh1": np.ascontiguousarray(whh_t[128:]).astype(MMNP),
        "wfc": np.ascontiguousarray(wfc_fold).astype(MMNP),
        "bias8": np.ascontiguousarray(bias.reshape(8, 128).T).astype(np.float32),
        "bfc": np.ascontiguousarray(b_fc.reshape(O, 1)).astype(np.float32),
    }
    in_maps = []
    for c in range(NCORES):
        xc = x[c * BL : (c + 1) * BL]  # [BL, S, F]
        xT = np.ascontiguousarray(xc.transpose(1, 2, 0)).astype(MMNP)  # [S, 32, BL]
        xrep = np.ascontiguousarray(np.tile(xT, (1, 4, 1)))  # [S, 128, BL]
        in_maps.append({"xrep": xrep, **shared})
    return in_maps


_NC_CACHE = {}


def _get_nc():
    if "nc" not in _NC_CACHE:
        _NC_CACHE["nc"] = build_nc()
    return _NC_CACHE["nc"]


def run(inputs, trace=False):
    in_maps = _host_prep(**inputs)
    nc = _get_nc()
    res = run_bass_kernel_spmd(nc, in_maps, core_ids=list(range(NCORES)), trace=trace)
    outs = []
    for c in range(NCORES):
        o = res.results[c]["out_t"]  # [33, 6, BL]
        outs.append(np.ascontiguousarray(o.transpose(2, 0, 1)))  # [BL, 33, 6]
    full = np.concatenate(outs, axis=0).astype(np.float32)  # [B, 33, 6]
    return full, res


def kernel(**inputs):
    full, _ = run(inputs, trace=False)
    return full


if __name__ == "__main__":
    t = build_nc()
    print("built ok")


# revision 16
# speedup vs baseline: 1.6060x; 1.0486x over previous
"""Trainium2 Bass kernel for nn_DynamicsLookAheadModel.

LSTM warm-up over S=96 steps + 32-step look-ahead with output feedback,
data-parallel over the batch (2048) across 8 NeuronCores (256 per core).

Per-core layout (all fp32):
  - Everything "transposed": hidden units on partitions, batch on the free dim.
    H=256 tensors are folded into [128, 512] tiles:
      phys[p, j]       = logical[p,     j]   for j <  256   (h-dims 0..127)
      phys[p, 256 + j] = logical[128+p, j]                  (h-dims 128..255)
  - Gates g = W_ih@x + W_hh@h computed on the PE into PSUM; the K=32
    x-projection runs as 4 concurrent quadrant matmuls (tile_position row
    packing, x and W_ih replicated across the 4 row bands), the K=256 h part
    as 2 full K=128 accumulation matmuls per M-tile.
  - PSUM: gate M-tile m (of 8) lives in bank m%4, columns 256*(m//4).
  - Bias (b_ih+b_hh) applied via the ScalarE activation bias port (per
    partition), one Sigmoid/Tanh instruction per M-tile from PSUM.
  - Cell update on VectorE in fp32.
  - STE binarization uses sign(c') (sigmoid(o) > 0, tanh sign-preserving):
    bits = (c' > 0) via one tensor_scalar is_gt.
  - Outputs are stored per step as [6, 256], assembled as [33, 6, 256] in
    DRAM; the host gather transposes to [256, 33, 6].
"""

import os

import numpy as np

import concourse.bass as bass
import concourse.mybir as mybir
import concourse.tile as tile
from concourse.bass_utils import run_bass_kernel_spmd

B, S, F, H, O = 2048, 96, 32, 256, 6
LA = 32
NCORES = 8
BL = B // NCORES  # 256 per-core batch
FD = 2 * BL  # 512: folded free dim for H=256 tensors
FP32 = mybir.dt.float32
# Matmul operand dtype: fp32 required — 16-bit quantization noise gets
# amplified by the STE binarization to several percent output error (measured).
MM16 = os.environ.get("MM16", "0") == "1"
MMDT = mybir.dt.float16 if MM16 else FP32
MMNP = np.float16 if MM16 else np.float32


# --- workaround: this walrus build allows only ONE sem wait per instruction ---
# Spill excess semaphore waits onto same-engine NOPs placed just before the
# instruction (engines execute their queue in order, so semantics match).
def _spill_excess_waits(nc, limit=1):
    cnt = 0
    for f in nc.m.functions:
        for bb in f.blocks:
            new_list = []
            for ins in bb.instructions:
                si = ins.sync_info
                if si and si.on_wait and len(si.on_wait) > limit:
                    waits = list(si.on_wait)
                    for w in waits[:-limit]:
                        n = mybir.InstNoOp(name=f"wspill_{cnt}", ins=[], outs=[])
                        cnt += 1
                        n.engine = ins.engine
                        n.sync_info = mybir.SyncInfo(on_wait=[w], on_update=[])
                        new_list.append(n)
                    ins.sync_info = mybir.SyncInfo(
                        on_wait=waits[-limit:], on_update=list(si.on_update)
                    )
                new_list.append(ins)
            bb.instructions[:] = new_list
    return cnt


def build_nc(n_warm=S, n_la=LA, spill=True):
    from contextlib import ExitStack

    nc = bass.Bass()
    AF = mybir.ActivationFunctionType
    ALU = mybir.AluOpType

    # xrep: x transposed per step and replicated over the 4 PE row bands.
    xrep_d = nc.dram_tensor("xrep", [n_warm, 128, BL], MMDT, kind="ExternalInput")
    # wrep: W_ih.T [32, 1024] replicated over the 4 row bands -> [128, 1024]
    wrep_d = nc.dram_tensor("wrep", [128, 4 * H], MMDT, kind="ExternalInput")
    whh0_d = nc.dram_tensor("whh0", [128, 4 * H], MMDT, kind="ExternalInput")
    whh1_d = nc.dram_tensor("whh1", [128, 4 * H], MMDT, kind="ExternalInput")
    wfc_d = nc.dram_tensor("wfc", [128, 2 * O], MMDT, kind="ExternalInput")
    bias8_d = nc.dram_tensor("bias8", [128, 8], FP32, kind="ExternalInput")
    bfc_d = nc.dram_tensor("bfc", [O, 1], FP32, kind="ExternalInput")
    out_d = nc.dram_tensor("out_t", [n_la + 1, O, BL], FP32, kind="ExternalOutput")

    with tile.TileContext(nc) as tc, ExitStack() as es:
        wp_ctx = es.enter_context(tc.tile_pool(name="weights", bufs=1))
        xp_ctx = es.enter_context(tc.tile_pool(name="xtiles", bufs=1))
        sp_ctx = es.enter_context(tc.tile_pool(name="state", bufs=2))
        gp_ctx = es.enter_context(tc.tile_pool(name="gates", bufs=1, space="PSUM"))

        wrep = wp_ctx.tile([128, 4 * H], MMDT, tag="wrep")
        nc.sync.dma_start(out=wrep, in_=wrep_d[:, :])
        whh0 = wp_ctx.tile([128, 4 * H], MMDT, tag="whh0")
        nc.sync.dma_start(out=whh0, in_=whh0_d[:, :])
        whh1 = wp_ctx.tile([128, 4 * H], MMDT, tag="whh1")
        nc.sync.dma_start(out=whh1, in_=whh1_d[:, :])
        wfc = wp_ctx.tile([128, 2 * O], MMDT, tag="wfc")
        nc.sync.dma_start(out=wfc, in_=wfc_d[:, :])
        bias8 = wp_ctx.tile([128, 8], FP32, tag="bias8")
        nc.sync.dma_start(out=bias8, in_=bias8_d[:, :])
        bfc = wp_ctx.tile([O, 1], FP32, tag="bfc")
        nc.sync.dma_start(out=bfc, in_=bfc_d[:, :])

        # x tiles (one per step; LA reuses tiles 0..31 with band rows 0:6
        # replaced by the fed-back output)
        xt = []
        for t in range(n_warm):
            xtile = xp_ctx.tile([128, BL], MMDT, tag=f"x{t}")
            nc.sync.dma_start(out=xtile, in_=xrep_d[t, :, :])
            xt.append(xtile)

        c_prev = None  # step 0 skips the f gate entirely
        h_prev = None  # step 0 skips the W_hh matmuls entirely

        # gate order i,f,g,o over M-tiles m=0..7 (gate X -> tiles 2X, 2X+1)
        # PSUM: one bank per M-tile (separate accumulation state per tile)
        GATE_FUNC = [AF.Sigmoid, AF.Sigmoid, AF.Tanh, AF.Sigmoid]  # i, f, g, o
        EMIT_ORDER = [2, 1, 0, 3]  # g, f, i, o

        def lstm_step(xtile, h_prev, c_prev, first=False):
            banks = []
            for b in range(8):
                pbank = gp_ctx.tile([128, BL], FP32, tag=f"pb{b}")
                banks.append(pbank)

            def psl(m):
                return banks[m][:, :]

            # x-projection: two quadrant-packed groups of 4 K=32 matmuls
            for grp in (0, 1):
                for band in range(4):
                    m = 4 * grp + band
                    if first and m in (2, 3):
                        continue  # f gate unused at step 0
                    nc.tensor.matmul(
                        psl(m),
                        wrep[32 * band : 32 * band + 32, 128 * m : 128 * m + 128],
                        xtile[32 * band : 32 * band + 32, :],
                        start=True,
                        stop=first,
                        tile_position=(32 * band, 0),
                        skip_group_check=True,
                    )
            # h part: per emission-ordered gate, 2 M-tiles x 2 K-tiles.
            # h is kept as two half tiles so k0 matmuls start as soon as the
            # low half of the tail finishes.
            if not first:
                for g in EMIT_ORDER:
                    for m in (2 * g, 2 * g + 1):
                        col = 128 * m
                        nc.tensor.matmul(
                            psl(m),
                            whh0[:, col : col + 128],
                            h_prev[0][:, :],
                            start=False,
                            stop=False,
                            skip_group_check=True,
                        )
                        nc.tensor.matmul(
                            psl(m),
                            whh1[:, col : col + 128],
                            h_prev[1][:, :],
                            start=False,
                            stop=True,
                            skip_group_check=True,
                        )

            # activations: one instr per M-tile into per-half tiles, bias via
            # the ACT bias port
            act = {}
            for g in EMIT_ORDER:
                if first and g == 1:
                    continue
                for half in (0, 1):
                    m = 2 * g + half
                    ah = sp_ctx.tile([128, BL], FP32, tag=f"a{g}_{half}")
                    act[(g, half)] = ah
                    nc.scalar.activation(
                        out=ah,
                        in_=psl(m),
                        func=GATE_FUNC[g],
                        bias=bias8[:, m : m + 1],
                    )

            # elementwise tail, low half first so h_lo lands early
            c_new = []
            h_new = []
            for half in (0, 1):
                cn = sp_ctx.tile([128, BL], FP32, tag=f"c{half}")
                if first:
                    nc.vector.tensor_tensor(
                        out=cn, in0=act[(0, half)], in1=act[(2, half)], op=ALU.mult
                    )
                else:
                    t1 = sp_ctx.tile([128, BL], FP32, tag=f"t1_{half}")
                    nc.vector.tensor_tensor(
                        out=t1, in0=act[(1, half)], in1=c_prev[half], op=ALU.mult
                    )
                    t2 = sp_ctx.tile([128, BL], FP32, tag=f"t2_{half}")
                    nc.vector.tensor_tensor(
                        out=t2, in0=act[(0, half)], in1=act[(2, half)], op=ALU.mult
                    )
                    nc.vector.tensor_tensor(out=cn, in0=t1, in1=t2, op=ALU.add)
                c_new.append(cn)
                tc_h = sp_ctx.tile([128, BL], FP32, tag=f"tc{half}")
                nc.scalar.activation(out=tc_h, in_=cn, func=AF.Tanh)
                hn = sp_ctx.tile([128, BL], MMDT, tag=f"h{half}")
                nc.vector.tensor_tensor(out=hn, in0=act[(3, half)], in1=tc_h, op=ALU.mult)
                h_new.append(hn)
            return h_new, c_new

        def emit_output(k, c_cur):
            # bits = (c' > 0); equals STE(h) since sigmoid(o)>0, tanh sign-pres.
            po = gp_ctx.tile([O, BL], FP32, tag="pb0")
            for half in (0, 1):
                bits = sp_ctx.tile([128, BL], MMDT, tag=f"bits{half}")
                nc.vector.tensor_scalar(
                    out=bits,
                    in0=c_cur[half],
                    scalar1=0.0,
                    scalar2=None,
                    op0=ALU.is_gt,
                )
                # po reuses a gate bank slot: all gate reads of this step are
                # done before bits is ready, so the WAR dep is satisfied
                nc.tensor.matmul(
                    po,
                    wfc[:, O * half : O * half + O],
                    bits[:, :],
                    start=(half == 0),
                    stop=(half == 1),
                    skip_group_check=True,
                )
            osb = sp_ctx.tile([O, BL], FP32, tag="osb")
            nc.scalar.activation(out=osb, in_=po, func=AF.Identity, bias=bfc)
            nc.sync.dma_start(out=out_d[k, :, :], in_=osb)
            return osb

        for t in range(n_warm):
            h_prev, c_prev = lstm_step(xt[t], h_prev, c_prev, first=(t == 0))

        for k in range(n_la + 1):
            osb = emit_output(k, c_prev)
            if k < n_la:
                for band in range(4):
                    nc.vector.tensor_copy(
                        out=xt[k][32 * band : 32 * band + O, :], in_=osb
                    )
                h_prev, c_prev = lstm_step(xt[k], h_prev, c_prev)

    if spill:
        _spill_excess_waits(nc)
    return nc


def _host_prep(x, W_ih, W_hh, b_ih, b_hh, W_fc, b_fc):
    """Build the 8 per-core input maps."""
    bias = (b_ih + b_hh).astype(np.float32)
    w32t = np.ascontiguousarray(W_ih.T).astype(MMNP)  # [32, 1024]
    wrep = np.ascontiguousarray(np.tile(w32t, (4, 1)))  # [128, 1024]
    whh_t = np.ascontiguousarray(W_hh.T)  # [256, 1024]
    wfc_fold = np.concatenate([W_fc.T[:128], W_fc.T[128:]], axis=1)  # [128, 12]
    shared = {
        "wrep": wrep,
        "whh0": np.ascontiguousarray(whh_t[:128]).astype(MMNP),
        "whh1": np.ascontiguousarray(whh_t[128:]).astype(MMNP),
        "wfc": np.ascontiguousarray(wfc_fold).astype(MMNP),
        "bias8": np.ascontiguousarray(bias.reshape(8, 128).T).astype(np.float32),
        "bfc": np.ascontiguousarray(b_fc.reshape(O, 1)).astype(np.float32),
    }
    in_maps = []
    for c in range(NCORES):
        xc = x[c * BL : (c + 1) * BL]  # [BL, S, F]
        xT = np.ascontiguousarray(xc.transpose(1, 2, 0)).astype(MMNP)  # [S, 32, BL]
        xrep = np.ascontiguousarray(np.tile(xT, (1, 4, 1)))  # [S, 128, BL]
        in_maps.append({"xrep": xrep, **shared})
    return in_maps


_NC_CACHE = {}


def _get_nc():
    if "nc" not in _NC_CACHE:
        _NC_CACHE["nc"] = build_nc()
    return _NC_CACHE["nc"]


def run(inputs, trace=False):
    in_maps = _host_prep(**inputs)
    nc = _get_nc()
    res = run_bass_kernel_spmd(nc, in_maps, core_ids=list(range(NCORES)), trace=trace)
    outs = []
    for c in range(NCORES):
        o = res.results[c]["out_t"]  # [33, 6, BL]
        outs.append(np.ascontiguousarray(o.transpose(2, 0, 1)))  # [BL, 33, 6]
    full = np.concatenate(outs, axis=0).astype(np.float32)  # [B, 33, 6]
    return full, res


def kernel(**inputs):
    full, _ = run(inputs, trace=False)
    return full


if __name__ == "__main__":
    t = build_nc()
    print("built ok")


# revision 18
# speedup vs baseline: 1.6452x; 1.0244x over previous
"""Trainium2 Bass kernel for nn_DynamicsLookAheadModel.

LSTM warm-up over S=96 steps + 32-step look-ahead with output feedback,
data-parallel over the batch (2048) across 8 NeuronCores (256 per core).

Per-core layout (all fp32):
  - Everything "transposed": hidden units on partitions, batch on the free dim.
    H=256 tensors are folded into [128, 512] tiles:
      phys[p, j]       = logical[p,     j]   for j <  256   (h-dims 0..127)
      phys[p, 256 + j] = logical[128+p, j]                  (h-dims 128..255)
  - Gates g = W_ih@x + W_hh@h computed on the PE into PSUM; the K=32
    x-projection runs as 4 concurrent quadrant matmuls (tile_position row
    packing, x and W_ih replicated across the 4 row bands), the K=256 h part
    as 2 full K=128 accumulation matmuls per M-tile.
  - PSUM: gate M-tile m (of 8) lives in bank m%4, columns 256*(m//4).
  - Bias (b_ih+b_hh) applied via the ScalarE activation bias port (per
    partition), one Sigmoid/Tanh instruction per M-tile from PSUM.
  - Cell update on VectorE in fp32.
  - STE binarization uses sign(c') (sigmoid(o) > 0, tanh sign-preserving):
    bits = (c' > 0) via one tensor_scalar is_gt.
  - Outputs are stored per step as [6, 256], assembled as [33, 6, 256] in
    DRAM; the host gather transposes to [256, 33, 6].
"""

import os

import numpy as np

import concourse.bass as bass
import concourse.mybir as mybir
import concourse.tile as tile
from concourse.bass_utils import run_bass_kernel_spmd

B, S, F, H, O = 2048, 96, 32, 256, 6
LA = 32
NCORES = 8
BL = B // NCORES  # 256 per-core batch
FD = 2 * BL  # 512: folded free dim for H=256 tensors
FP32 = mybir.dt.float32
# Matmul operand dtype: fp32 required — 16-bit quantization noise gets
# amplified by the STE binarization to several percent output error (measured).
MM16 = os.environ.get("MM16", "0") == "1"
MMDT = mybir.dt.float16 if MM16 else FP32
MMNP = np.float16 if MM16 else np.float32


# --- workaround: this walrus build allows only ONE sem wait per instruction ---
# Spill excess semaphore waits onto same-engine NOPs placed just before the
# instruction (engines execute their queue in order, so semantics match).
def _spill_excess_waits(nc, limit=1):
    cnt = 0
    for f in nc.m.functions:
        for bb in f.blocks:
            new_list = []
            for ins in bb.instructions:
                si = ins.sync_info
                if si and si.on_wait and len(si.on_wait) > limit:
                    waits = list(si.on_wait)
                    for w in waits[:-limit]:
                        n = mybir.InstNoOp(name=f"wspill_{cnt}", ins=[], outs=[])
                        cnt += 1
                        n.engine = ins.engine
                        n.sync_info = mybir.SyncInfo(on_wait=[w], on_update=[])
                        new_list.append(n)
                    ins.sync_info = mybir.SyncInfo(
                        on_wait=waits[-limit:], on_update=list(si.on_update)
                    )
                new_list.append(ins)
            bb.instructions[:] = new_list
    return cnt


def build_nc(n_warm=S, n_la=LA, spill=True):
    from contextlib import ExitStack

    nc = bass.Bass()
    AF = mybir.ActivationFunctionType
    ALU = mybir.AluOpType

    # xrep: x transposed, step-PAIRED on the free dim (N=512 per pair), and
    # replicated over the 4 PE row bands.
    assert n_warm % 2 == 0
    xrep_d = nc.dram_tensor(
        "xrep", [n_warm // 2, 128, 2 * BL], MMDT, kind="ExternalInput"
    )
    # wrep: W_ih.T [32, 1024] replicated over the 4 row bands -> [128, 1024]
    wrep_d = nc.dram_tensor("wrep", [128, 4 * H], MMDT, kind="ExternalInput")
    whh0_d = nc.dram_tensor("whh0", [128, 4 * H], MMDT, kind="ExternalInput")
    whh1_d = nc.dram_tensor("whh1", [128, 4 * H], MMDT, kind="ExternalInput")
    wfc_d = nc.dram_tensor("wfc", [128, 2 * O], MMDT, kind="ExternalInput")
    bias8_d = nc.dram_tensor("bias8", [128, 8], FP32, kind="ExternalInput")
    bfc_d = nc.dram_tensor("bfc", [O, 1], FP32, kind="ExternalInput")
    out_d = nc.dram_tensor("out_t", [n_la + 1, O, BL], FP32, kind="ExternalOutput")

    with tile.TileContext(nc) as tc, ExitStack() as es:
        wp_ctx = es.enter_context(tc.tile_pool(name="weights", bufs=1))
        xp_ctx = es.enter_context(tc.tile_pool(name="xtiles", bufs=1))
        sp_ctx = es.enter_context(tc.tile_pool(name="state", bufs=2))
        gp_ctx = es.enter_context(tc.tile_pool(name="gates", bufs=1, space="PSUM"))

        wrep = wp_ctx.tile([128, 4 * H], MMDT, tag="wrep")
        nc.sync.dma_start(out=wrep, in_=wrep_d[:, :])
        whh0 = wp_ctx.tile([128, 4 * H], MMDT, tag="whh0")
        nc.sync.dma_start(out=whh0, in_=whh0_d[:, :])
        whh1 = wp_ctx.tile([128, 4 * H], MMDT, tag="whh1")
        nc.sync.dma_start(out=whh1, in_=whh1_d[:, :])
        wfc = wp_ctx.tile([128, 2 * O], MMDT, tag="wfc")
        nc.sync.dma_start(out=wfc, in_=wfc_d[:, :])
        bias8 = wp_ctx.tile([128, 8], FP32, tag="bias8")
        nc.sync.dma_start(out=bias8, in_=bias8_d[:, :])
        bfc = wp_ctx.tile([O, 1], FP32, tag="bfc")
        nc.sync.dma_start(out=bfc, in_=bfc_d[:, :])

        # x pair tiles; per-step views slice the pair's half (LA reuses the
        # views for steps 0..31 with band rows 0:6 replaced by the output)
        xpt = []
        for p in range(n_warm // 2):
            xtile = xp_ctx.tile([128, 2 * BL], MMDT, tag=f"x{p}")
            nc.sync.dma_start(out=xtile, in_=xrep_d[p, :, :])
            xpt.append(xtile)

        def xt_view(t):
            return xpt[t // 2][:, BL * (t % 2) : BL * (t % 2) + BL]

        c_prev = None  # step 0 skips the f gate entirely
        h_prev = None  # step 0 skips the W_hh matmuls entirely

        # gate order i,f,g,o over M-tiles m=0..7 (gate X -> tiles 2X, 2X+1)
        # PSUM: one bank per M-tile (separate accumulation state per tile)
        GATE_FUNC = [AF.Sigmoid, AF.Sigmoid, AF.Tanh, AF.Sigmoid]  # i, f, g, o
        EMIT_ORDER = [2, 1, 0, 3]  # g, f, i, o

        def alloc_banks():
            banks = []
            for b in range(8):
                pbank = gp_ctx.tile([128, FD], FP32, tag=f"pb{b}")
                banks.append(pbank)
            return banks

        def gates_pair(xptile):
            # x-projection for TWO steps at once (N=512), quadrant-packed
            banks = alloc_banks()
            for grp in (0, 1):
                for band in range(4):
                    m = 4 * grp + band
                    nc.tensor.matmul(
                        banks[m][:, :],
                        wrep[32 * band : 32 * band + 32, 128 * m : 128 * m + 128],
                        xptile[32 * band : 32 * band + 32, :],
                        start=True,
                        stop=False,
                        tile_position=(32 * band, 0),
                        skip_group_check=True,
                    )
            return banks

        def gates_single(xslice, first=False):
            banks = alloc_banks()
            for grp in (0, 1):
                for band in range(4):
                    m = 4 * grp + band
                    if first and m in (2, 3):
                        continue  # f gate unused at step 0
                    nc.tensor.matmul(
                        banks[m][:, 0:BL],
                        wrep[32 * band : 32 * band + 32, 128 * m : 128 * m + 128],
                        xslice[32 * band : 32 * band + 32, :],
                        start=True,
                        stop=first,
                        tile_position=(32 * band, 0),
                        skip_group_check=True,
                    )
            return banks

        def lstm_tail(banks, off, h_prev, c_prev, first=False):
            def psl(m):
                return banks[m][:, off : off + BL]

            # h part: per emission-ordered gate, 2 M-tiles x 2 K-tiles.
            # h lives in two half tiles so k0 matmuls start as soon as the
            # low half of the tail finishes.
            if not first:
                for g in EMIT_ORDER:
                    for m in (2 * g, 2 * g + 1):
                        col = 128 * m
                        nc.tensor.matmul(
                            psl(m),
                            whh0[:, col : col + 128],
                            h_prev[0][:, :],
                            start=False,
                            stop=False,
                            skip_group_check=True,
                        )
                        nc.tensor.matmul(
                            psl(m),
                            whh1[:, col : col + 128],
                            h_prev[1][:, :],
                            start=False,
                            stop=True,
                            skip_group_check=True,
                        )

            # activations: one instr per M-tile into per-half tiles, bias via
            # the ACT bias port
            act = {}
            for g in EMIT_ORDER:
                if first and g == 1:
                    continue
                for half in (0, 1):
                    m = 2 * g + half
                    ah = sp_ctx.tile([128, BL], FP32, tag=f"a{g}_{half}")
                    act[(g, half)] = ah
                    nc.scalar.activation(
                        out=ah,
                        in_=psl(m),
                        func=GATE_FUNC[g],
                        bias=bias8[:, m : m + 1],
                    )

            # elementwise tail, low half first so h_lo lands early
            c_new = []
            h_new = []
            for half in (0, 1):
                cn = sp_ctx.tile([128, BL], FP32, tag=f"c{half}")
                if first:
                    nc.vector.tensor_tensor(
                        out=cn, in0=act[(0, half)], in1=act[(2, half)], op=ALU.mult
                    )
                else:
                    t1 = sp_ctx.tile([128, BL], FP32, tag=f"t1_{half}")
                    nc.vector.tensor_tensor(
                        out=t1, in0=act[(1, half)], in1=c_prev[half], op=ALU.mult
                    )
                    t2 = sp_ctx.tile([128, BL], FP32, tag=f"t2_{half}")
                    nc.vector.tensor_tensor(
                        out=t2, in0=act[(0, half)], in1=act[(2, half)], op=ALU.mult
                    )
                    nc.vector.tensor_tensor(out=cn, in0=t1, in1=t2, op=ALU.add)
                c_new.append(cn)
                tc_h = sp_ctx.tile([128, BL], FP32, tag=f"tc{half}")
                nc.scalar.activation(out=tc_h, in_=cn, func=AF.Tanh)
                hn = sp_ctx.tile([128, BL], MMDT, tag=f"h{half}")
                nc.vector.tensor_tensor(out=hn, in0=act[(3, half)], in1=tc_h, op=ALU.mult)
                h_new.append(hn)
            return h_new, c_new

        def emit_output(k, c_cur):
            # bits = (c' > 0); equals STE(h) since sigmoid(o)>0, tanh sign-pres.
            po = gp_ctx.tile([O, BL], FP32, tag="pb0")
            for half in (0, 1):
                bits = sp_ctx.tile([128, BL], MMDT, tag=f"bits{half}")
                nc.vector.tensor_scalar(
                    out=bits,
                    in0=c_cur[half],
                    scalar1=0.0,
                    scalar2=None,
                    op0=ALU.is_gt,
                )
                # po reuses a gate bank slot: all gate reads of this step are
                # done before bits is ready, so the WAR dep is satisfied
                nc.tensor.matmul(
                    po,
                    wfc[:, O * half : O * half + O],
                    bits[:, :],
                    start=(half == 0),
                    stop=(half == 1),
                    skip_group_check=True,
                )
            osb = sp_ctx.tile([O, BL], FP32, tag="osb")
            nc.scalar.activation(out=osb, in_=po, func=AF.Identity, bias=bfc)
            nc.sync.dma_start(out=out_d[k, :, :], in_=osb)
            return osb

        # steps 0 and 1 unpaired (step 0 has no h part), pairs from step 2
        bk = gates_single(xt_view(0), first=True)
        h_prev, c_prev = lstm_tail(bk, 0, None, None, first=True)
        bk = gates_single(xt_view(1))
        h_prev, c_prev = lstm_tail(bk, 0, h_prev, c_prev)
        for p in range(1, n_warm // 2):
            bk = gates_pair(xpt[p])
            h_prev, c_prev = lstm_tail(bk, 0, h_prev, c_prev)
            h_prev, c_prev = lstm_tail(bk, BL, h_prev, c_prev)

        for k in range(n_la + 1):
            osb = emit_output(k, c_prev)
            if k < n_la:
                xv = xt_view(k)
                for band in range(4):
                    nc.vector.tensor_copy(
                        out=xv[32 * band : 32 * band + O, :], in_=osb
                    )
                bk = gates_single(xv)
                h_prev, c_prev = lstm_tail(bk, 0, h_prev, c_prev)

    if spill:
        _spill_excess_waits(nc)
    return nc


def _host_prep(x, W_ih, W_hh, b_ih, b_hh, W_fc, b_fc):
    """Build the 8 per-core input maps."""
    bias = (b_ih + b_hh).astype(np.float32)
    w32t = np.ascontiguousarray(W_ih.T).astype(MMNP)  # [32, 1024]
    wrep = np.ascontiguousarray(np.tile(w32t, (4, 1)))  # [128, 1024]
    whh_t = np.ascontiguousarray(W_hh.T)  # [256, 1024]
    wfc_fold = np.concatenate([W_fc.T[:128], W_fc.T[128:]], axis=1)  # [128, 12]
    shared = {
        "wrep": wrep,
        "whh0": np.ascontiguousarray(whh_t[:128]).astype(MMNP),
        "whh1": np.ascontiguousarray(whh_t[128:]).astype(MMNP),
        "wfc": np.ascontiguousarray(wfc_fold).astype(MMNP),
        "bias8": np.ascontiguousarray(bias.reshape(8, 128).T).astype(np.float32),
        "bfc": np.ascontiguousarray(b_fc.reshape(O, 1)).astype(np.float32),
    }
    in_maps = []
    for c in range(NCORES):
        xc = x[c * BL : (c + 1) * BL]  # [BL, S, F]
        xT = np.ascontiguousarray(xc.transpose(1, 2, 0)).astype(MMNP)  # [S, 32, BL]
        xpair = (
            xT.reshape(S // 2, 2, F, BL).transpose(0, 2, 1, 3).reshape(S // 2, F, 2 * BL)
        )
        xrep = np.ascontiguousarray(np.tile(xpair, (1, 4, 1)))  # [S/2, 128, 2BL]
        in_maps.append({"xrep": xrep, **shared})
    return in_maps


_NC_CACHE = {}


def _get_nc():
    if "nc" not in _NC_CACHE:
        _NC_CACHE["nc"] = build_nc()
    return _NC_CACHE["nc"]


def run(inputs, trace=False):
    in_maps = _host_prep(**inputs)
    nc = _get_nc()
    res = run_bass_kernel_spmd(nc, in_maps, core_ids=list(range(NCORES)), trace=trace)
    outs = []
    for c in range(NCORES):
        o = res.results[c]["out_t"]  # [33, 6, BL]
        outs.append(np.ascontiguousarray(o.transpose(2, 0, 1)))  # [BL, 33, 6]
    full = np.concatenate(outs, axis=0).astype(np.float32)  # [B, 33, 6]
    return full, res


def kernel(**inputs):
    full, _ = run(inputs, trace=False)
    return full


if __name__ == "__main__":
    t = build_nc()
    print("built ok")
